# revision 30
# baseline (speedup 1.0000x reference)
# CondConv2d Trainium2 kernel.
#
# Math (per sample n=(b,l)):
#   pooled[c]   = mean_{h,w} x[n,c,h,w]
#   allxet      = [p0,p0,p0,p1,p2,p3] temporal window (first frame dup'd twice)
#   calib[c,t]  = conv1d(allxet, tconv_w)[c,t] + tconv_b[c]
#   gate[t]     = conv1d(allxet, fc_w)[0,t] + fc_b
#   scale[n,c]  = calib[c,l] + 1
#   out[n,o]    = conv2d(x[n] * scale[n,:,None,None], weight) + bias[o]*(gate[l]+1)
# (the per-sample weight scale fw = weight * scale[n,ci] is folded into the
#  input because conv is linear in each input channel)
#
# Sharding: data-parallel over b: 8 cores x 2 samples. Weights replicated.
# Conv as implicit GEMM in bf16 (weights + scaled activations), fp32 PSUM
# accumulate: contraction over ci (2 chunks of 128 partitions), 9
# shifted-window matmuls per chunk. bf16 weights get fast-weight-load so
# the per-matmul LDWEIGHTS hides under the previous matmul's streaming.
#
# Schedule: per-frame software pipeline, emission order == per-engine
# queue order (all queues are in-order):
#  - every input/output DMA rides the Sync engine's hardware-dynamic
#    queue (~0.6us issue vs ~2.7us software descriptor gen on GpSimd),
#    ordered so frame 0 + the weights it needs land first
#  - x / out are host-repacked partition-major so each frame is one
#    contiguous-per-partition fused DMA
#  - a short burst of zero "warm-up" matmuls runs while frame 0 is in
#    flight so the PE HAM clock-gate reaches 2.4 GHz before the real
#    stream starts
#  - the small calib/gate matmuls are interleaved between conv
#    accumulation groups at points where their data is already resident
#  - the last frame's epilogue is split per PSUM bank half so the final
#    bias-add/DMA overlaps the last matmul group

import numpy as np
import ml_dtypes


def _install_axon_ntff_shim():
    # This container's `antenv` stub lacks `axon_hooks`, which
    # bass_utils imports unconditionally when trace=True under axon.
    # Provide it (and register the ctypes NTFF hook if the .so is
    # present) so tracing works; missing pieces degrade to no-trace.
    import os
    import sys
    import types

    try:
        import antenv.axon_hooks  # noqa: F401

        return
    except Exception:
        pass
    try:
        import antenv
    except Exception:
        return
    mod = types.ModuleType("antenv.axon_hooks")
    mod._hook = None

    def set_axon_ntff_profile_hook(h):
        mod._hook = h

    def get_axon_ntff_profile_hook():
        return mod._hook

    mod.set_axon_ntff_profile_hook = set_axon_ntff_profile_hook
    mod.get_axon_ntff_profile_hook = get_axon_ntff_profile_hook
    sys.modules["antenv.axon_hooks"] = mod
    antenv.axon_hooks = mod
    try:
        from trn_agent_boot.trn_boot import _ntff_profile_via_ctypes

        so = "/opt/axon/libaxon_pjrt.so"
        if os.path.exists(so):
            mod._hook = _ntff_profile_via_ctypes(so)
    except Exception:
        pass


_install_axon_ntff_shim()

import concourse.bass as bass
import concourse.tile as tile
from concourse import mybir
from concourse.bass_utils import run_bass_kernel_spmd

B, L, CIN, COUT, KS, H, W = 16, 4, 256, 256, 3, 32, 32
NCORES = 8
BS = B // NCORES      # batch samples per core
NF = BS * L           # frames per core
CC = CIN // 128       # ci chunks
OC = COUT // 128      # co chunks
WP = W + 2            # x tile row width incl. zero pad cols
HW = H * W
FP32 = mybir.dt.float32
BF16 = mybir.dt.bfloat16
HHALF = 16            # psum bank = 512 fp32 = 16 rows of 32
NWARM = 10            # HAM warm-up matmuls (~4.3us at the cold clock)
NWARM2 = 6            # second warm-up burst bridging the calib->conv gap

_last_results = None  # test harness reads exec_time_ns from here


def _split_excess_waits(nc):
    # walrus in this toolchain encodes exactly one sem wait per engine
    # instruction (TPB_EVENTS has a single wait slot) and optimize_sems
    # is disabled, so Tile can emit instructions with >1 wait that fail
    # codegen ("Too many sync wait commands").  Split the excess waits
    # into standalone EventSemaphore instructions on the same engine
    # stream immediately before the instruction; in-order issue makes
    # this equivalent.  Applies to Drain too (CTRL struct: one wait).
    n = 0
    f = nc.m.functions[0]
    for bb in f.blocks:
        insts = list(bb.instructions)
        out = []
        changed = False
        for inst in insts:
            si = inst.sync_info
            if si is not None:
                waits = list(si.on_wait)
                # matmuls carry ZERO waits: a standalone ES ahead of the
                # LDWEIGHTS/MATMUL pair is processed by the NX while the
                # previous matmul is still streaming, so a pre-satisfied
                # wait costs nothing instead of ~200ns at the group start
                limit = 0 if isinstance(inst, mybir.InstMatmult) else 1
                if len(waits) > limit:
                    keep = waits[len(waits) - limit:] if limit else []
                    for w in waits[:len(waits) - limit]:
                        n += 1
                        es = mybir.InstEventSemaphore(name=f"ES-SPLIT-{n}")
                        es.engine = inst.engine
                        es.sync_info = mybir.SyncInfo(on_wait=[w], on_update=[])
                        out.append(es)
                    si.on_wait = keep
                    inst.sync_info = si
                    changed = True
            out.append(inst)
        if changed:
            bb.instructions = out
    return n


def build_nc():
    nc = bass.Bass()
    # x / out are host-repacked partition-major: [BS, 128p, L, CC, H, W]
    x_d = nc.dram_tensor("x", [BS, 128, L, CC, H, W], FP32, kind="ExternalInput")
    w0_d = nc.dram_tensor("w0", [128, CC, 9, 128], BF16, kind="ExternalInput")
    w1_d = nc.dram_tensor("w1", [128, CC, 9, 128], BF16, kind="ExternalInput")
    # sum of the three taps (for l=0, where all taps read the dup'd frame)
    tcw3_d = nc.dram_tensor("tconv3", [128, CC, CIN], BF16, kind="ExternalInput")
    tcw_d = nc.dram_tensor("tconv", [128, CC, 3, CIN], BF16, kind="ExternalInput")
    fcw_d = nc.dram_tensor("fc", [128, CC, 3], BF16, kind="ExternalInput")
    bias_d = nc.dram_tensor("bias2", [128, OC], FP32, kind="ExternalInput")
    tb1_d = nc.dram_tensor("tb1", [128, CC], FP32, kind="ExternalInput")
    fcb1_d = nc.dram_tensor("fcb1", [1, 1], FP32, kind="ExternalInput")
    out_d = nc.dram_tensor("out", [BS, L, 128, OC, H, W], FP32, kind="ExternalOutput")

    with tile.TileContext(nc) as tc:
        with (
            # bf16 pooled sums / gate: their quantization contributes ~2e-4
            # relative output error vs the 2e-2 tolerance
            nc.allow_low_precision(reason="bf16 matmul path, tolerance 2e-2"),
            tc.tile_pool(name="singles", bufs=1) as singles,
            tc.tile_pool(name="outp", bufs=3) as outp,
            tc.tile_pool(name="pp_conv", bufs=3, space="PSUM") as pp_conv,
            tc.tile_pool(name="pp_small", bufs=2, space="PSUM") as pp_small,
        ):
            # ---- persistent tiles ----
            w_sb = [
                singles.tile([128, CC, 9, 128], BF16, tag="w0", name="w0_sb"),
                singles.tile([128, CC, 9, 128], BF16, tag="w1", name="w1_sb"),
            ]
            tcw3_sb = singles.tile([128, CC, CIN], BF16, tag="tcw3")
            tcw_sb = singles.tile([128, CC, 3, CIN], BF16, tag="tcw")
            fcw_sb = singles.tile([128, CC, 3], BF16, tag="fcw")
            bias_sb = singles.tile([128, OC], FP32, tag="bias")
            tb1_sb = singles.tile([128, CC], FP32, tag="tb1")
            fcb1_sb = singles.tile([1, 1], FP32, tag="fcb1")
            ones_sb = singles.tile([1, 128], BF16, tag="ones")
            zcol_sb = singles.tile([128, H, 1], FP32, tag="zcol")
            warm_w = singles.tile([128, 128], BF16, tag="warm_w")
            warm_x = singles.tile([128, 512], BF16, tag="warm_x")
            pool_scr = singles.tile([128, H, W], FP32, tag="poolscr")
            pool_acc = singles.tile([128, 1], FP32, tag="poolacc")
            allxet = singles.tile([128, CC, BS, L + 2], BF16, tag="allxet")
            s_sb = singles.tile([128, CC, BS, L], FP32, tag="s")
            g1_sb = singles.tile([1, BS, L], BF16, tag="g1")
            fb_sb = singles.tile([128, BS, L, OC], FP32, tag="fb")
            # raw x staging: one resident buffer for the whole core slice
            xbig = singles.tile([128, BS, L, CC, H, W], FP32, tag="xbig")
            # bf16 conv-input tiles (zero-pad cols written per-frame)
            x_t = {}
            for f in range(NF):
                for ci in range(CC):
                    x_t[(f, ci)] = singles.tile(
                        [128, H, WP], BF16, tag=f"xt{f}_{ci}", name=f"xt{f}_{ci}"
                    )

            # warm-up operand memsets first: they gate the warm-up matmuls
            nc.vector.memset(warm_w[:], 0.0)
            nc.vector.memset(warm_x[:], 0.0)
            nc.vector.memset(ones_sb[:], 1.0)
            nc.vector.memset(zcol_sb[:], 0.0)

            def zpad(f, defer_ms=None):
                # zero the two pad columns of frame f's conv-input tiles.
                # defer_ms pushes them past the startup-critical DVE window
                # (they have no data deps, so the scheduler would otherwise
                # hoist all of them ahead of frame 0's pooling reduce)
                import contextlib

                ctx = (
                    tc.tile_wait_until(defer_ms)
                    if defer_ms is not None
                    else contextlib.nullcontext()
                )
                with ctx:
                    for ci in range(CC):
                        nc.vector.tensor_copy(x_t[(f, ci)][:, :, 0:1], zcol_sb[:])
                        nc.vector.tensor_copy(x_t[(f, ci)][:, :, WP - 1:WP], zcol_sb[:])

            zpad(0)

            # ---- DMAs: all on the Sync hardware-dynamic queue, in
            # priority order (single queue -> deterministic arrival) ----
            nc.sync.dma_start(out=xbig[:, 0, 0, 0], in_=x_d[0, :, 0, 0])
            nc.sync.dma_start(out=xbig[:, 0, 0, 1], in_=x_d[0, :, 0, 1])
            nc.sync.dma_start(out=tcw3_sb[:], in_=tcw3_d[:])
            nc.sync.dma_start(out=tb1_sb[:], in_=tb1_d[:])
            nc.sync.dma_start(out=w_sb[0][:], in_=w0_d[:])
            nc.sync.dma_start(out=tcw_sb[:], in_=tcw_d[:])
            nc.sync.dma_start(out=fcw_sb[:], in_=fcw_d[:])
            nc.sync.dma_start(out=bias_sb[:], in_=bias_d[:])
            nc.sync.dma_start(out=fcb1_sb[:], in_=fcb1_d[:])
            nc.sync.dma_start(out=w_sb[1][:], in_=w1_d[:])
            nc.sync.dma_start(out=xbig[:, 0, 1], in_=x_d[0, :, 1])
            nc.sync.dma_start(out=xbig[:, 0, 2:4], in_=x_d[0, :, 2:4])
            nc.sync.dma_start(out=xbig[:, 1], in_=x_d[1, :])

            # ---- PE clock warm-up: zero matmuls while frame 0 is in
            # flight; HAM un-throttles to 2.4 GHz after ~3.4us busy.
            # warm_ps lives in pp_conv so later pp_small tiles don't
            # inherit a write-after-write ordering on the warm bank ----
            warm_ps = pp_conv.tile([128, 512], FP32, tag="convps", name="warm_ps")
            for _ in range(NWARM):
                nc.tensor.matmul(
                    warm_ps[:, :], lhsT=warm_w[:], rhs=warm_x[:],
                    start=True, stop=True,
                )

            # ---- helpers ----
            def pool_frame(f, act_ci1=False):
                # act_ci1 (frame 0 only): ci1's pooling rides the Scalar
                # engine via activation-accumulate, so a straggling ci0
                # DMA semaphore can't serialize the two reduces on DVE
                b, l = divmod(f, L)
                nc.vector.reduce_sum(
                    out=allxet[:, 0, b, 2 + l:3 + l],
                    in_=xbig[:, b, l, 0],
                    axis=mybir.AxisListType.XY,
                )
                if act_ci1:
                    nc.scalar.activation(
                        out=pool_scr[:],
                        in_=xbig[:, b, l, 1],
                        func=mybir.ActivationFunctionType.Copy,
                        accum_out=pool_acc[:, 0:1],
                    )
                    nc.scalar.copy(allxet[:, 1, b, 2 + l:3 + l], pool_acc[:, 0:1])
                else:
                    nc.vector.reduce_sum(
                        out=allxet[:, 1, b, 2 + l:3 + l],
                        in_=xbig[:, b, l, 1],
                        axis=mybir.AxisListType.XY,
                    )

            def dup_first(b):
                for ci in range(CC):
                    nc.vector.tensor_copy(allxet[:, ci, b, 0:1], allxet[:, ci, b, 2:3])
                    nc.vector.tensor_copy(allxet[:, ci, b, 1:2], allxet[:, ci, b, 2:3])

            def calib_cols(b, l0, n, sum_tap=False):
                # scale[:, l] = calib[:, l] + tconv_b + 1 for l in [l0, l0+n)
                for oc in range(OC):
                    pc = pp_small.tile([128, L], FP32, tag="smallpsum")
                    if sum_tap:
                        mms = [(ci, -1, 2) for ci in range(CC)]
                    else:
                        mms = [(ci, k, k + l0) for ci in range(CC) for k in range(3)]
                    for i, (ci, k, c0) in enumerate(mms):
                        lhsT = (
                            tcw3_sb[:, ci, oc * 128:(oc + 1) * 128]
                            if k < 0
                            else tcw_sb[:, ci, k, oc * 128:(oc + 1) * 128]
                        )
                        nc.tensor.matmul(
                            pc[:, 0:n],
                            lhsT=lhsT,
                            rhs=allxet[:, ci, b, c0:c0 + n],
                            start=(i == 0),
                            stop=(i == len(mms) - 1),
                        )
                    nc.vector.tensor_scalar_add(
                        s_sb[:, oc, b, l0:l0 + n], pc[:, 0:n], tb1_sb[:, oc:oc + 1]
                    )

            def gate_b(b):
                # fb[:, b, l, oc] = bias * (gate[l] + fc_b + 1)
                pg = pp_small.tile([128, L], FP32, tag="smallpsum")
                mms = [(ci, k) for ci in range(CC) for k in range(3)]
                for i, (ci, k) in enumerate(mms):
                    nc.tensor.matmul(
                        pg[0:1, 0:L],
                        lhsT=fcw_sb[:, ci, k:k + 1],
                        rhs=allxet[:, ci, b, k:k + L],
                        start=(i == 0),
                        stop=(i == len(mms) - 1),
                    )
                nc.vector.tensor_scalar_add(
                    g1_sb[0:1, b, :], pg[0:1, 0:L], fcb1_sb[0:1, 0:1]
                )
                gb = pp_small.tile([128, L], FP32, tag="smallpsum")
                nc.tensor.matmul(
                    gb[:, 0:L], lhsT=ones_sb[0:1, :], rhs=g1_sb[0:1, b, :],
                    start=True, stop=True,
                )
                for l in range(L):
                    for oc in range(OC):
                        nc.vector.tensor_mul(
                            fb_sb[:, b, l, oc:oc + 1],
                            gb[:, l:l + 1],
                            bias_sb[:, oc:oc + 1],
                        )

            def scale_frame(f, split=False):
                # x_t = x * scale; doubles as the fp32->bf16 rounding op.
                # split=True (startup critical path) runs ci0 on DVE and
                # ci1 on ACT concurrently, each in two row-halves so the
                # conv's first accumulation group (rows 0..16) unblocks
                # after the first half; otherwise both chunks ride ACT.
                b, l = divmod(f, L)
                if split:
                    for rA, rB in ((0, HHALF + 1), (HHALF + 1, H)):
                        for ci in range(CC):
                            args = (
                                x_t[(f, ci)][:, rA:rB, 1:W + 1],
                                xbig[:, b, l, ci, rA:rB],
                                s_sb[:, ci, b, l:l + 1],
                            )
                            if ci == 0:
                                nc.vector.tensor_scalar_mul(*args)
                            else:
                                nc.scalar.mul(*args)
                else:
                    for ci in range(CC):
                        nc.scalar.mul(
                            x_t[(f, ci)][:, :, 1:W + 1],
                            xbig[:, b, l, ci],
                            s_sb[:, ci, b, l:l + 1],
                        )

            def conv_mm(f, oc):
                # one (frame, oc-chunk) implicit-GEMM accumulation: 36 MMs
                ps = pp_conv.tile([128, H, W], FP32, tag="convps")
                for half in range(H // HHALF):
                    h0 = half * HHALF
                    group = []
                    # kh=1 first: the group's start matmul then covers the
                    # full 16-row bank (full has_written clear; also keeps
                    # CoreSim's bank-granular pending-zero model happy)
                    for ci in range(CC):
                        for kh in (1, 0, 2):
                            dh = kh - 1
                            hA = max(h0, -dh)
                            hB = min(h0 + HHALF, H - dh)
                            if hB <= hA:
                                continue
                            for kw in range(3):
                                group.append((ci, kh, kw, hA, hB))
                    for i, (ci, kh, kw, hA, hB) in enumerate(group):
                        dh = kh - 1
                        nc.tensor.matmul(
                            ps[:, hA:hB, :],
                            lhsT=w_sb[oc][:, ci, kh * 3 + kw, :],
                            rhs=x_t[(f, ci)][:, hA + dh:hB + dh, kw:kw + W],
                            start=(i == 0),
                            stop=(i == len(group) - 1),
                        )
                return ps

            def finish(f, psA, psB):
                # fused per-frame epilogue: bias-add both oc chunks into one
                # staging tile, single output DMA
                b, l = divmod(f, L)
                osb = outp.tile([128, OC, H, W], FP32, tag="osb")
                nc.vector.tensor_scalar_add(osb[:, 0], psA[:], fb_sb[:, b, l, 0:1])
                nc.vector.tensor_scalar_add(osb[:, 1], psB[:], fb_sb[:, b, l, 1:2])
                nc.sync.dma_start(out=out_d[b, l], in_=osb[:])

            def finish_last(f, psA, psB):
                # per-oc tail: oc0's bias-add + DMA complete while oc1's
                # matmul group is still streaming; oc-contiguous DMA slices
                # keep 4KB-per-partition packets (row-sliced DMAs degrade
                # to 2KB packets at ~140ns/packet)
                b, l = divmod(f, L)
                osb = outp.tile([128, OC, H, W], FP32, tag="osb")
                nc.vector.tensor_scalar_add(osb[:, 0], psA[:], fb_sb[:, b, l, 0:1])
                # oc1's bias-add split per PSUM bank: the bank-0 half (DVE)
                # overlaps the bank-1 matmuls still streaming; the bank-1
                # half rides ACT so only ~0.7us trails the last matmul.
                # Both DMAs go out on the Scalar engine's HW-dynamic queue:
                # the final one then issues back-to-back with the ACT
                # bias-add instead of paying a cross-queue semaphore hop.
                nc.scalar.dma_start(out=out_d[b, l, :, 0:1], in_=osb[:, 0:1])
                nc.vector.tensor_scalar_add(
                    osb[:, 1, 0:HHALF], psB[:, 0:HHALF], fb_sb[:, b, l, 1:2]
                )
                nc.scalar.add(
                    osb[:, 1, HHALF:H], psB[:, HHALF:H], fb_sb[:, b, l, 1:2]
                )
                nc.scalar.dma_start(out=out_d[b, l, :, 1:2], in_=osb[:, 1:2])

            def warm_burst(n):
                for _ in range(n):
                    nc.tensor.matmul(
                        warm_ps[:, :], lhsT=warm_w[:], rhs=warm_x[:],
                        start=True, stop=True,
                    )

            # ---- schedule ----
            # frame 0 critical chain: reduce -> calib(sum-tap; needs no
            # dup) -> scale (DVE+ACT split). Warm matmuls are interleaved
            # into every dependency wait so the PE has no idle window
            # before the conv stream (a ~2us idle re-throttles the HAM
            # clock gate to 1.2 GHz, which then costs ~3us of cold conv).
            pool_frame(0, act_ci1=True)
            # calib l0, ci-chunks interleaved with warm fill: the ci0
            # matmuls only wait on ci0's pooling, which lands ~1.3us
            # before ci1's
            pc0 = []
            for oc in range(OC):
                pc = pp_small.tile([128, L], FP32, tag="smallpsum", name=f"pc0_{oc}")
                pc0.append(pc)
                nc.tensor.matmul(
                    pc[:, 0:1],
                    lhsT=tcw3_sb[:, 0, oc * 128:(oc + 1) * 128],
                    rhs=allxet[:, 0, 0, 2:3],
                    start=True, stop=False,
                )
            warm_burst(4)
            for oc in range(OC):
                nc.tensor.matmul(
                    pc0[oc][:, 0:1],
                    lhsT=tcw3_sb[:, 1, oc * 128:(oc + 1) * 128],
                    rhs=allxet[:, 1, 0, 2:3],
                    start=False, stop=True,
                )
                nc.vector.tensor_scalar_add(
                    s_sb[:, oc, 0, 0:1], pc0[oc][:, 0:1], tb1_sb[:, oc:oc + 1]
                )
            warm_burst(NWARM2)
            scale_frame(0, split=True)
            dup_first(0)

            ps00 = conv_mm(0, 0)
            # calib for l=1 lands here: frame 1 is pooled by the time the
            # first conv group drains
            pool_frame(1)
            calib_cols(0, 1, 1)
            zpad(1, defer_ms=0.012)
            scale_frame(1)
            pool_frame(2)
            pool_frame(3)
            ps01 = conv_mm(0, 1)
            # rest of sample 0: l=2,3 + gate; then frame 0 can be finished
            calib_cols(0, 2, 2)
            gate_b(0)
            finish(0, ps00, ps01)
            zpad(2, defer_ms=0.016)
            scale_frame(2)
            zpad(3, defer_ms=0.016)
            scale_frame(3)

            ps10 = conv_mm(1, 0)
            for f in range(L, NF):
                pool_frame(f)
            dup_first(1)
            ps11 = conv_mm(1, 1)
            # sample 1 prologue: all of b=1's frames are resident well
            # before these matmuls reach the queue head
            calib_cols(1, 0, 4)
            gate_b(1)
            finish(1, ps10, ps11)
            for f in range(L, NF):
                zpad(f, defer_ms=0.024)
                scale_frame(f)

            for f in range(2, NF):
                psA = conv_mm(f, 0)
                psB = conv_mm(f, 1)
                if f == NF - 1:
                    finish_last(f, psA, psB)
                else:
                    finish(f, psA, psB)

    return nc


def pack_inputs(x, weight, bias, tconv_w, tconv_b, fc_w, fc_b):
    x = np.asarray(x, dtype=np.float32)
    weight = np.asarray(weight, dtype=np.float32)
    bias = np.asarray(bias, dtype=np.float32)
    tconv_w = np.asarray(tconv_w, dtype=np.float32)
    tconv_b = np.asarray(tconv_b, dtype=np.float32)
    fc_w = np.asarray(fc_w, dtype=np.float32)
    fc_b = np.asarray(fc_b, dtype=np.float32)
    bf16 = ml_dtypes.bfloat16

    w_host = (
        weight.transpose(1, 2, 3, 0).reshape(CC, 128, 9, COUT).transpose(1, 0, 2, 3)
    ).astype(bf16)
    w0_host = np.ascontiguousarray(w_host[:, :, :, 0:128])
    w1_host = np.ascontiguousarray(w_host[:, :, :, 128:COUT])
    # fold the 1/(H*W) pooling normalization into the conv1d weights
    tcw = (tconv_w / HW).transpose(1, 2, 0).reshape(CC, 128, 3, CIN)
    tcw_host = np.ascontiguousarray(tcw.transpose(1, 0, 2, 3)).astype(bf16)
    tcw3_host = np.ascontiguousarray(
        tcw.sum(axis=2).transpose(1, 0, 2)
    ).astype(bf16)
    fcw_host = np.ascontiguousarray(
        (fc_w[0] / HW).reshape(CC, 128, 3).transpose(1, 0, 2)
    ).astype(bf16)
    bias_host = np.ascontiguousarray(bias.reshape(OC, 128).T)
    tb1_host = np.ascontiguousarray((tconv_b + 1.0).reshape(CC, 128).T)
    fcb1_host = np.ascontiguousarray((fc_b + 1.0).reshape(1, 1))

    # partition-major x: [BS, 128p, L, CC, H, W]
    xp = x.reshape(B, L, CC, 128, H, W).transpose(0, 3, 1, 2, 4, 5)

    in_maps = []
    for core in range(NCORES):
        in_maps.append({
            "x": np.ascontiguousarray(xp[core * BS:(core + 1) * BS]),
            "w0": w0_host,
            "w1": w1_host,
            "tconv3": tcw3_host,
            "tconv": tcw_host,
            "fc": fcw_host,
            "bias2": bias_host,
            "tb1": tb1_host,
            "fcb1": fcb1_host,
        })
    return in_maps


def unpack_output(res_out):
    # [BS, L, 128p, OC, H, W] -> [BS*L, COUT, H, W]
    return np.ascontiguousarray(
        res_out.transpose(0, 1, 3, 2, 4, 5).reshape(BS * L, COUT, H, W)
    )


def kernel(x, weight, bias, tconv_w, tconv_b, fc_w, fc_b):
    global _last_results
    in_maps = pack_inputs(x, weight, bias, tconv_w, tconv_b, fc_w, fc_b)
    nc = build_nc()
    # walrus codegen needs <=1 sem wait per instruction; CoreSim's race
    # detector chokes on the split ES instructions, so only split for HW
    _split_excess_waits(nc)
    res = run_bass_kernel_spmd(nc, in_maps, core_ids=list(range(NCORES)))
    _last_results = res
    out = np.concatenate([unpack_output(r["out"]) for r in res.results], axis=0)
    return out


# revision 31
# speedup vs baseline: 1.0247x; 1.0247x over previous
# CondConv2d Trainium2 kernel.
#
# Math (per sample n=(b,l)):
#   pooled[c]   = mean_{h,w} x[n,c,h,w]
#   allxet      = [p0,p0,p0,p1,p2,p3] temporal window (first frame dup'd twice)
#   calib[c,t]  = conv1d(allxet, tconv_w)[c,t] + tconv_b[c]
#   gate[t]     = conv1d(allxet, fc_w)[0,t] + fc_b
#   scale[n,c]  = calib[c,l] + 1
#   out[n,o]    = conv2d(x[n] * scale[n,:,None,None], weight) + bias[o]*(gate[l]+1)
# (the per-sample weight scale fw = weight * scale[n,ci] is folded into the
#  input because conv is linear in each input channel)
#
# Sharding: data-parallel over b: 8 cores x 2 samples. Weights replicated.
# Conv as implicit GEMM in bf16 (weights + scaled activations), fp32 PSUM
# accumulate: contraction over ci (2 chunks of 128 partitions), 9
# shifted-window matmuls per chunk. bf16 weights get fast-weight-load so
# the per-matmul LDWEIGHTS hides under the previous matmul's streaming.
#
# Schedule: per-frame software pipeline, emission order == per-engine
# queue order (all queues are in-order):
#  - every input/output DMA rides the Sync engine's hardware-dynamic
#    queue (~0.6us issue vs ~2.7us software descriptor gen on GpSimd),
#    ordered so frame 0 + the weights it needs land first
#  - x / out are host-repacked partition-major so each frame is one
#    contiguous-per-partition fused DMA
#  - a short burst of zero "warm-up" matmuls runs while frame 0 is in
#    flight so the PE HAM clock-gate reaches 2.4 GHz before the real
#    stream starts
#  - the small calib/gate matmuls are interleaved between conv
#    accumulation groups at points where their data is already resident
#  - the last frame's epilogue is split per PSUM bank half so the final
#    bias-add/DMA overlaps the last matmul group

import numpy as np
import ml_dtypes


def _install_axon_ntff_shim():
    # This container's `antenv` stub lacks `axon_hooks`, which
    # bass_utils imports unconditionally when trace=True under axon.
    # Provide it (and register the ctypes NTFF hook if the .so is
    # present) so tracing works; missing pieces degrade to no-trace.
    import os
    import sys
    import types

    try:
        import antenv.axon_hooks  # noqa: F401

        return
    except Exception:
        pass
    try:
        import antenv
    except Exception:
        return
    mod = types.ModuleType("antenv.axon_hooks")
    mod._hook = None

    def set_axon_ntff_profile_hook(h):
        mod._hook = h

    def get_axon_ntff_profile_hook():
        return mod._hook

    mod.set_axon_ntff_profile_hook = set_axon_ntff_profile_hook
    mod.get_axon_ntff_profile_hook = get_axon_ntff_profile_hook
    sys.modules["antenv.axon_hooks"] = mod
    antenv.axon_hooks = mod
    try:
        from trn_agent_boot.trn_boot import _ntff_profile_via_ctypes

        so = "/opt/axon/libaxon_pjrt.so"
        if os.path.exists(so):
            mod._hook = _ntff_profile_via_ctypes(so)
    except Exception:
        pass


_install_axon_ntff_shim()

import concourse.bass as bass
import concourse.tile as tile
from concourse import mybir
from concourse.bass_utils import run_bass_kernel_spmd

B, L, CIN, COUT, KS, H, W = 16, 4, 256, 256, 3, 32, 32
NCORES = 8
BS = B // NCORES      # batch samples per core
NF = BS * L           # frames per core
CC = CIN // 128       # ci chunks
OC = COUT // 128      # co chunks
WP = W + 2            # x tile row width incl. zero pad cols
HW = H * W
FP32 = mybir.dt.float32
BF16 = mybir.dt.bfloat16
HHALF = 16            # psum bank = 512 fp32 = 16 rows of 32
NWARM = 10            # HAM warm-up matmuls (~4.3us at the cold clock)
NWARM2 = 6            # second warm-up burst bridging the calib->conv gap

_last_results = None  # test harness reads exec_time_ns from here


def _split_excess_waits(nc):
    # walrus in this toolchain encodes exactly one sem wait per engine
    # instruction (TPB_EVENTS has a single wait slot) and optimize_sems
    # is disabled, so Tile can emit instructions with >1 wait that fail
    # codegen ("Too many sync wait commands").  Split the excess waits
    # into standalone EventSemaphore instructions on the same engine
    # stream immediately before the instruction; in-order issue makes
    # this equivalent.  Applies to Drain too (CTRL struct: one wait).
    n = 0
    f = nc.m.functions[0]
    for bb in f.blocks:
        insts = list(bb.instructions)
        out = []
        changed = False
        for inst in insts:
            si = inst.sync_info
            if si is not None:
                waits = list(si.on_wait)
                if len(waits) > 1:
                    for w in waits[:-1]:
                        n += 1
                        es = mybir.InstEventSemaphore(name=f"ES-SPLIT-{n}")
                        es.engine = inst.engine
                        es.sync_info = mybir.SyncInfo(on_wait=[w], on_update=[])
                        out.append(es)
                    si.on_wait = [waits[-1]]
                    inst.sync_info = si
                    changed = True
            out.append(inst)
        if changed:
            bb.instructions = out
    return n


def build_nc():
    nc = bass.Bass()
    # x / out are host-repacked partition-major: [BS, 128p, L, CC, H, W]
    x_d = nc.dram_tensor("x", [BS, 128, L, CC, H, W], FP32, kind="ExternalInput")
    w0_d = nc.dram_tensor("w0", [128, CC, 9, 128], BF16, kind="ExternalInput")
    w1_d = nc.dram_tensor("w1", [128, CC, 9, 128], BF16, kind="ExternalInput")
    # sum of the three taps (for l=0, where all taps read the dup'd frame)
    tcw3_d = nc.dram_tensor("tconv3", [128, CC, CIN], BF16, kind="ExternalInput")
    tcw_d = nc.dram_tensor("tconv", [128, CC, 3, CIN], BF16, kind="ExternalInput")
    fcw_d = nc.dram_tensor("fc", [128, CC, 3], BF16, kind="ExternalInput")
    bias_d = nc.dram_tensor("bias2", [128, OC], FP32, kind="ExternalInput")
    tb1_d = nc.dram_tensor("tb1", [128, CC], FP32, kind="ExternalInput")
    fcb1_d = nc.dram_tensor("fcb1", [1, 1], FP32, kind="ExternalInput")
    out_d = nc.dram_tensor("out", [BS, L, 128, OC, H, W], FP32, kind="ExternalOutput")

    with tile.TileContext(nc) as tc:
        with (
            # bf16 pooled sums / gate: their quantization contributes ~2e-4
            # relative output error vs the 2e-2 tolerance
            nc.allow_low_precision(reason="bf16 matmul path, tolerance 2e-2"),
            tc.tile_pool(name="singles", bufs=1) as singles,
            tc.tile_pool(name="outp", bufs=3) as outp,
            tc.tile_pool(name="pp_conv", bufs=3, space="PSUM") as pp_conv,
            tc.tile_pool(name="pp_small", bufs=2, space="PSUM") as pp_small,
        ):
            # ---- persistent tiles ----
            w_sb = [
                singles.tile([128, CC, 9, 128], BF16, tag="w0", name="w0_sb"),
                singles.tile([128, CC, 9, 128], BF16, tag="w1", name="w1_sb"),
            ]
            tcw3_sb = singles.tile([128, CC, CIN], BF16, tag="tcw3")
            tcw_sb = singles.tile([128, CC, 3, CIN], BF16, tag="tcw")
            fcw_sb = singles.tile([128, CC, 3], BF16, tag="fcw")
            bias_sb = singles.tile([128, OC], FP32, tag="bias")
            tb1_sb = singles.tile([128, CC], FP32, tag="tb1")
            fcb1_sb = singles.tile([1, 1], FP32, tag="fcb1")
            ones_sb = singles.tile([1, 128], BF16, tag="ones")
            zcol_sb = singles.tile([128, H, 1], FP32, tag="zcol")
            warm_w = singles.tile([128, 128], BF16, tag="warm_w")
            warm_x = singles.tile([128, 512], BF16, tag="warm_x")
            pool_scr = singles.tile([128, H, W], FP32, tag="poolscr")
            pool_acc = singles.tile([128, 1], FP32, tag="poolacc")
            allxet = singles.tile([128, CC, BS, L + 2], BF16, tag="allxet")
            s_sb = singles.tile([128, CC, BS, L], FP32, tag="s")
            g1_sb = singles.tile([1, BS, L], BF16, tag="g1")
            fb_sb = singles.tile([128, BS, L, OC], FP32, tag="fb")
            # raw x staging: one resident buffer for the whole core slice
            xbig = singles.tile([128, BS, L, CC, H, W], FP32, tag="xbig")
            # bf16 conv-input tiles (zero-pad cols written per-frame)
            x_t = {}
            for f in range(NF):
                for ci in range(CC):
                    x_t[(f, ci)] = singles.tile(
                        [128, H, WP], BF16, tag=f"xt{f}_{ci}", name=f"xt{f}_{ci}"
                    )

            # warm-up operand memsets first: they gate the warm-up matmuls
            nc.vector.memset(warm_w[:], 0.0)
            nc.vector.memset(warm_x[:], 0.0)
            nc.vector.memset(ones_sb[:], 1.0)
            nc.vector.memset(zcol_sb[:], 0.0)

            def zpad(f, defer_ms=None):
                # zero the two pad columns of frame f's conv-input tiles.
                # defer_ms pushes them past the startup-critical DVE window
                # (they have no data deps, so the scheduler would otherwise
                # hoist all of them ahead of frame 0's pooling reduce)
                import contextlib

                ctx = (
                    tc.tile_wait_until(defer_ms)
                    if defer_ms is not None
                    else contextlib.nullcontext()
                )
                with ctx:
                    for ci in range(CC):
                        nc.vector.tensor_copy(x_t[(f, ci)][:, :, 0:1], zcol_sb[:])
                        nc.vector.tensor_copy(x_t[(f, ci)][:, :, WP - 1:WP], zcol_sb[:])

            zpad(0)

            # ---- DMAs: all on the Sync hardware-dynamic queue, in
            # priority order (single queue -> deterministic arrival) ----
            nc.sync.dma_start(out=xbig[:, 0, 0, 0], in_=x_d[0, :, 0, 0])
            nc.sync.dma_start(out=xbig[:, 0, 0, 1], in_=x_d[0, :, 0, 1])
            nc.sync.dma_start(out=tcw3_sb[:], in_=tcw3_d[:])
            nc.sync.dma_start(out=tb1_sb[:], in_=tb1_d[:])
            nc.sync.dma_start(out=w_sb[0][:], in_=w0_d[:])
            nc.sync.dma_start(out=tcw_sb[:], in_=tcw_d[:])
            nc.sync.dma_start(out=fcw_sb[:], in_=fcw_d[:])
            nc.sync.dma_start(out=bias_sb[:], in_=bias_d[:])
            nc.sync.dma_start(out=fcb1_sb[:], in_=fcb1_d[:])
            nc.sync.dma_start(out=w_sb[1][:], in_=w1_d[:])
            nc.sync.dma_start(out=xbig[:, 0, 1], in_=x_d[0, :, 1])
            nc.sync.dma_start(out=xbig[:, 0, 2:4], in_=x_d[0, :, 2:4])
            nc.sync.dma_start(out=xbig[:, 1], in_=x_d[1, :])

            # ---- PE clock warm-up: zero matmuls while frame 0 is in
            # flight; HAM un-throttles to 2.4 GHz after ~3.4us busy.
            # warm_ps lives in pp_conv so later pp_small tiles don't
            # inherit a write-after-write ordering on the warm bank ----
            warm_ps = pp_conv.tile([128, 512], FP32, tag="convps", name="warm_ps")
            for _ in range(NWARM):
                nc.tensor.matmul(
                    warm_ps[:, :], lhsT=warm_w[:], rhs=warm_x[:],
                    start=True, stop=True,
                )

            # ---- helpers ----
            def pool_frame(f, act_ci1=False):
                # act_ci1 (frame 0 only): ci1's pooling rides the Scalar
                # engine via activation-accumulate, so a straggling ci0
                # DMA semaphore can't serialize the two reduces on DVE
                b, l = divmod(f, L)
                nc.vector.reduce_sum(
                    out=allxet[:, 0, b, 2 + l:3 + l],
                    in_=xbig[:, b, l, 0],
                    axis=mybir.AxisListType.XY,
                )
                if act_ci1:
                    nc.scalar.activation(
                        out=pool_scr[:],
                        in_=xbig[:, b, l, 1],
                        func=mybir.ActivationFunctionType.Copy,
                        accum_out=pool_acc[:, 0:1],
                    )
                    nc.scalar.copy(allxet[:, 1, b, 2 + l:3 + l], pool_acc[:, 0:1])
                else:
                    nc.vector.reduce_sum(
                        out=allxet[:, 1, b, 2 + l:3 + l],
                        in_=xbig[:, b, l, 1],
                        axis=mybir.AxisListType.XY,
                    )

            def dup_first(b):
                for ci in range(CC):
                    nc.vector.tensor_copy(allxet[:, ci, b, 0:1], allxet[:, ci, b, 2:3])
                    nc.vector.tensor_copy(allxet[:, ci, b, 1:2], allxet[:, ci, b, 2:3])

            def calib_cols(b, l0, n, sum_tap=False):
                # scale[:, l] = calib[:, l] + tconv_b + 1 for l in [l0, l0+n)
                for oc in range(OC):
                    pc = pp_small.tile([128, L], FP32, tag="smallpsum")
                    if sum_tap:
                        mms = [(ci, -1, 2) for ci in range(CC)]
                    else:
                        mms = [(ci, k, k + l0) for ci in range(CC) for k in range(3)]
                    for i, (ci, k, c0) in enumerate(mms):
                        lhsT = (
                            tcw3_sb[:, ci, oc * 128:(oc + 1) * 128]
                            if k < 0
                            else tcw_sb[:, ci, k, oc * 128:(oc + 1) * 128]
                        )
                        nc.tensor.matmul(
                            pc[:, 0:n],
                            lhsT=lhsT,
                            rhs=allxet[:, ci, b, c0:c0 + n],
                            start=(i == 0),
                            stop=(i == len(mms) - 1),
                        )
                    nc.vector.tensor_scalar_add(
                        s_sb[:, oc, b, l0:l0 + n], pc[:, 0:n], tb1_sb[:, oc:oc + 1]
                    )

            def gate_b(b):
                # fb[:, b, l, oc] = bias * (gate[l] + fc_b + 1)
                pg = pp_small.tile([128, L], FP32, tag="smallpsum")
                mms = [(ci, k) for ci in range(CC) for k in range(3)]
                for i, (ci, k) in enumerate(mms):
                    nc.tensor.matmul(
                        pg[0:1, 0:L],
                        lhsT=fcw_sb[:, ci, k:k + 1],
                        rhs=allxet[:, ci, b, k:k + L],
                        start=(i == 0),
                        stop=(i == len(mms) - 1),
                    )
                nc.vector.tensor_scalar_add(
                    g1_sb[0:1, b, :], pg[0:1, 0:L], fcb1_sb[0:1, 0:1]
                )
                gb = pp_small.tile([128, L], FP32, tag="smallpsum")
                nc.tensor.matmul(
                    gb[:, 0:L], lhsT=ones_sb[0:1, :], rhs=g1_sb[0:1, b, :],
                    start=True, stop=True,
                )
                for l in range(L):
                    for oc in range(OC):
                        nc.vector.tensor_mul(
                            fb_sb[:, b, l, oc:oc + 1],
                            gb[:, l:l + 1],
                            bias_sb[:, oc:oc + 1],
                        )

            def scale_frame(f, split=False):
                # x_t = x * scale; doubles as the fp32->bf16 rounding op.
                # split=True (startup critical path) runs ci0 on DVE and
                # ci1 on ACT concurrently, each in two row-halves so the
                # conv's first accumulation group (rows 0..16) unblocks
                # after the first half; otherwise both chunks ride ACT.
                b, l = divmod(f, L)
                if split:
                    for rA, rB in ((0, HHALF + 1), (HHALF + 1, H)):
                        for ci in range(CC):
                            args = (
                                x_t[(f, ci)][:, rA:rB, 1:W + 1],
                                xbig[:, b, l, ci, rA:rB],
                                s_sb[:, ci, b, l:l + 1],
                            )
                            if ci == 0:
                                nc.vector.tensor_scalar_mul(*args)
                            else:
                                nc.scalar.mul(*args)
                else:
                    for ci in range(CC):
                        nc.scalar.mul(
                            x_t[(f, ci)][:, :, 1:W + 1],
                            xbig[:, b, l, ci],
                            s_sb[:, ci, b, l:l + 1],
                        )

            def conv_mm(f, oc):
                # one (frame, oc-chunk) implicit-GEMM accumulation: 36 MMs
                ps = pp_conv.tile([128, H, W], FP32, tag="convps")
                for half in range(H // HHALF):
                    h0 = half * HHALF
                    group = []
                    # kh=1 first: the group's start matmul then covers the
                    # full 16-row bank (full has_written clear; also keeps
                    # CoreSim's bank-granular pending-zero model happy)
                    for ci in range(CC):
                        for kh in (1, 0, 2):
                            dh = kh - 1
                            hA = max(h0, -dh)
                            hB = min(h0 + HHALF, H - dh)
                            if hB <= hA:
                                continue
                            for kw in range(3):
                                group.append((ci, kh, kw, hA, hB))
                    for i, (ci, kh, kw, hA, hB) in enumerate(group):
                        dh = kh - 1
                        nc.tensor.matmul(
                            ps[:, hA:hB, :],
                            lhsT=w_sb[oc][:, ci, kh * 3 + kw, :],
                            rhs=x_t[(f, ci)][:, hA + dh:hB + dh, kw:kw + W],
                            start=(i == 0),
                            stop=(i == len(group) - 1),
                        )
                return ps

            def finish(f, psA, psB):
                # fused per-frame epilogue: bias-add both oc chunks into one
                # staging tile, single output DMA
                b, l = divmod(f, L)
                osb = outp.tile([128, OC, H, W], FP32, tag="osb")
                nc.vector.tensor_scalar_add(osb[:, 0], psA[:], fb_sb[:, b, l, 0:1])
                nc.vector.tensor_scalar_add(osb[:, 1], psB[:], fb_sb[:, b, l, 1:2])
                nc.sync.dma_start(out=out_d[b, l], in_=osb[:])

            def finish_last(f, psA, psB):
                # per-oc tail: oc0's bias-add + DMA complete while oc1's
                # matmul group is still streaming; oc-contiguous DMA slices
                # keep 4KB-per-partition packets (row-sliced DMAs degrade
                # to 2KB packets at ~140ns/packet)
                b, l = divmod(f, L)
                osb = outp.tile([128, OC, H, W], FP32, tag="osb")
                nc.vector.tensor_scalar_add(osb[:, 0], psA[:], fb_sb[:, b, l, 0:1])
                # oc1's bias-add split per PSUM bank: the bank-0 half (DVE)
                # overlaps the bank-1 matmuls still streaming; the bank-1
                # half rides ACT so only ~0.7us trails the last matmul.
                # Both DMAs go out on the Scalar engine's HW-dynamic queue:
                # the final one then issues back-to-back with the ACT
                # bias-add instead of paying a cross-queue semaphore hop.
                nc.scalar.dma_start(out=out_d[b, l, :, 0:1], in_=osb[:, 0:1])
                nc.vector.tensor_scalar_add(
                    osb[:, 1, 0:HHALF], psB[:, 0:HHALF], fb_sb[:, b, l, 1:2]
                )
                nc.scalar.add(
                    osb[:, 1, HHALF:H], psB[:, HHALF:H], fb_sb[:, b, l, 1:2]
                )
                nc.scalar.dma_start(out=out_d[b, l, :, 1:2], in_=osb[:, 1:2])

            def warm_burst(n):
                for _ in range(n):
                    nc.tensor.matmul(
                        warm_ps[:, :], lhsT=warm_w[:], rhs=warm_x[:],
                        start=True, stop=True,
                    )

            # ---- schedule ----
            # frame 0 critical chain: reduce -> calib(sum-tap; needs no
            # dup) -> scale (DVE+ACT split). Warm matmuls are interleaved
            # into every dependency wait so the PE has no idle window
            # before the conv stream (a ~2us idle re-throttles the HAM
            # clock gate to 1.2 GHz, which then costs ~3us of cold conv).
            pool_frame(0, act_ci1=True)
            # calib l0, ci-chunks interleaved with warm fill: the ci0
            # matmuls only wait on ci0's pooling, which lands ~1.3us
            # before ci1's
            pc0 = []
            for oc in range(OC):
                pc = pp_small.tile([128, L], FP32, tag="smallpsum", name=f"pc0_{oc}")
                pc0.append(pc)
                nc.tensor.matmul(
                    pc[:, 0:1],
                    lhsT=tcw3_sb[:, 0, oc * 128:(oc + 1) * 128],
                    rhs=allxet[:, 0, 0, 2:3],
                    start=True, stop=False,
                )
            warm_burst(4)
            for oc in range(OC):
                nc.tensor.matmul(
                    pc0[oc][:, 0:1],
                    lhsT=tcw3_sb[:, 1, oc * 128:(oc + 1) * 128],
                    rhs=allxet[:, 1, 0, 2:3],
                    start=False, stop=True,
                )
                nc.vector.tensor_scalar_add(
                    s_sb[:, oc, 0, 0:1], pc0[oc][:, 0:1], tb1_sb[:, oc:oc + 1]
                )
            warm_burst(NWARM2)
            scale_frame(0, split=True)
            dup_first(0)

            ps00 = conv_mm(0, 0)
            # calib for l=1 lands here: frame 1 is pooled by the time the
            # first conv group drains
            pool_frame(1)
            calib_cols(0, 1, 1)
            zpad(1, defer_ms=0.012)
            scale_frame(1)
            pool_frame(2)
            pool_frame(3)
            ps01 = conv_mm(0, 1)
            # rest of sample 0: l=2,3 + gate; then frame 0 can be finished
            calib_cols(0, 2, 2)
            gate_b(0)
            finish(0, ps00, ps01)
            zpad(2, defer_ms=0.016)
            scale_frame(2)
            zpad(3, defer_ms=0.016)
            scale_frame(3)

            ps10 = conv_mm(1, 0)
            for f in range(L, NF):
                pool_frame(f)
            dup_first(1)
            ps11 = conv_mm(1, 1)
            # sample 1 prologue: all of b=1's frames are resident well
            # before these matmuls reach the queue head
            calib_cols(1, 0, 4)
            gate_b(1)
            finish(1, ps10, ps11)
            for f in range(L, NF):
                zpad(f, defer_ms=0.024)
                scale_frame(f)

            for f in range(2, NF):
                psA = conv_mm(f, 0)
                psB = conv_mm(f, 1)
                if f == NF - 1:
                    finish_last(f, psA, psB)
                else:
                    finish(f, psA, psB)

    return nc


def pack_inputs(x, weight, bias, tconv_w, tconv_b, fc_w, fc_b):
    x = np.asarray(x, dtype=np.float32)
    weight = np.asarray(weight, dtype=np.float32)
    bias = np.asarray(bias, dtype=np.float32)
    tconv_w = np.asarray(tconv_w, dtype=np.float32)
    tconv_b = np.asarray(tconv_b, dtype=np.float32)
    fc_w = np.asarray(fc_w, dtype=np.float32)
    fc_b = np.asarray(fc_b, dtype=np.float32)
    bf16 = ml_dtypes.bfloat16

    w_host = (
        weight.transpose(1, 2, 3, 0).reshape(CC, 128, 9, COUT).transpose(1, 0, 2, 3)
    ).astype(bf16)
    w0_host = np.ascontiguousarray(w_host[:, :, :, 0:128])
    w1_host = np.ascontiguousarray(w_host[:, :, :, 128:COUT])
    # fold the 1/(H*W) pooling normalization into the conv1d weights
    tcw = (tconv_w / HW).transpose(1, 2, 0).reshape(CC, 128, 3, CIN)
    tcw_host = np.ascontiguousarray(tcw.transpose(1, 0, 2, 3)).astype(bf16)
    tcw3_host = np.ascontiguousarray(
        tcw.sum(axis=2).transpose(1, 0, 2)
    ).astype(bf16)
    fcw_host = np.ascontiguousarray(
        (fc_w[0] / HW).reshape(CC, 128, 3).transpose(1, 0, 2)
    ).astype(bf16)
    bias_host = np.ascontiguousarray(bias.reshape(OC, 128).T)
    tb1_host = np.ascontiguousarray((tconv_b + 1.0).reshape(CC, 128).T)
    fcb1_host = np.ascontiguousarray((fc_b + 1.0).reshape(1, 1))

    # partition-major x: [BS, 128p, L, CC, H, W]
    xp = x.reshape(B, L, CC, 128, H, W).transpose(0, 3, 1, 2, 4, 5)

    in_maps = []
    for core in range(NCORES):
        in_maps.append({
            "x": np.ascontiguousarray(xp[core * BS:(core + 1) * BS]),
            "w0": w0_host,
            "w1": w1_host,
            "tconv3": tcw3_host,
            "tconv": tcw_host,
            "fc": fcw_host,
            "bias2": bias_host,
            "tb1": tb1_host,
            "fcb1": fcb1_host,
        })
    return in_maps


def unpack_output(res_out):
    # [BS, L, 128p, OC, H, W] -> [BS*L, COUT, H, W]
    return np.ascontiguousarray(
        res_out.transpose(0, 1, 3, 2, 4, 5).reshape(BS * L, COUT, H, W)
    )


def kernel(x, weight, bias, tconv_w, tconv_b, fc_w, fc_b):
    global _last_results
    in_maps = pack_inputs(x, weight, bias, tconv_w, tconv_b, fc_w, fc_b)
    nc = build_nc()
    # walrus codegen needs <=1 sem wait per instruction; CoreSim's race
    # detector chokes on the split ES instructions, so only split for HW
    _split_excess_waits(nc)
    res = run_bass_kernel_spmd(nc, in_maps, core_ids=list(range(NCORES)))
    _last_results = res
    out = np.concatenate([unpack_output(r["out"]) for r in res.results], axis=0)
    return out
